# revision 1
# baseline (speedup 1.0000x reference)
"""Trainium2 Bass kernel for the GNN message-update MLP:

    out = relu(concat([v_i, v_j, e_ij], -1) @ W1 + b1) @ W2 + b2

Strategy (memory-bound, E = 1M edges, data-parallel across 8 cores):
  - Shard edges across the 8 NeuronCores (125000 each, padded to 126976).
  - Split-precision fp16 matmuls: every fp32 activation x ships as an
    (fp16 hi, fp16 lo) pair — same bytes as fp32 — and each logical fp32
    matmul x@w becomes xh@wh + xh@wl + xl@wh (the dropped lo@lo term is
    ~2^-22 relative). This runs the PE at full rate (1 cyc/row vs the 4x
    slower 2-pass fp32 mode) with ~5e-7 relative error vs the fp32 ref.
  - The three e_ij correction matmuls are K-stacked ([eh; eh; el], K=96)
    into ONE matmul per tile: 7 matmuls per 512-edge tile, all N=512.
  - Per 512-edge tile q (parity alternates PE column groups so output
    DMAs span all 128 partitions):
      layer1: 3x K=128 matmul + 1x K=96 e-matmul -> PSUM
      VectorE: hh = fp16(relu(psum + b1))   (tensor_scalar from PSUM)
      ScalarE: t  = relu(psum + b1)  fp32
      VectorE: hl = t - hh           fp16
      layer2: hh@w2h + hh@w2l + hl@w2h      -> PSUM
      ScalarE: out = psum (plain copy; b2 is added on host)
  - Host pre-packs transposed layouts so the device does only
    contiguous full-partition DMAs.
"""

import numpy as np

import concourse.bacc as bacc
import concourse.bass as bass
import concourse.mybir as mybir
import concourse.tile as tile
from concourse.bass_utils import run_bass_kernel_spmd

# ---- problem constants (hardcoded per harness contract) ----
E_TOTAL = 1_000_000
N_CORES = 8
IN_C = 64
IN_E = 32
HID = 64
OUT_C = 64

NHALF = 512                    # edges per matmul (moving free dim, 1 psum bank)
Q_PER_BLK = 8                  # 512-edge tiles per block
BLK_EDGES = NHALF * Q_PER_BLK  # 4096
EPC = E_TOTAL // N_CORES       # 125000 edges per core
N_BLK = -(-EPC // BLK_EDGES)   # 31
EPAD = N_BLK * BLK_EDGES       # 126976

import os
import ml_dtypes

_HALF = os.environ.get("KERNEL_HALF", "fp16")
F32 = mybir.dt.float32
F16 = mybir.dt.bfloat16 if _HALF == "bf16" else mybir.dt.float16
_NP_HALF = ml_dtypes.bfloat16 if _HALF == "bf16" else np.float16

# test.py hooks
_TRACE = False
LAST_RESULT = None

_PROGRAM_CACHE = {}


def _build_program():
    nc = bacc.Bacc(
        "TRN2",
        target_bir_lowering=False,
        debug=False,
        num_devices=N_CORES,
    )

    xta = nc.declare_dram_parameter(
        "xta", [N_BLK, 128, 2, BLK_EDGES], F16, isOutput=False
    )
    xtb = nc.declare_dram_parameter(
        "xtb", [N_BLK, 96, Q_PER_BLK, NHALF], F16, isOutput=False
    )
    w1a_h = nc.declare_dram_parameter("w1a_h", [128, HID], F16, isOutput=False)
    w1a_l = nc.declare_dram_parameter("w1a_l", [128, HID], F16, isOutput=False)
    w_es = nc.declare_dram_parameter("w_es", [96, HID], F16, isOutput=False)
    w2h_r = nc.declare_dram_parameter("w2h_r", [128, OUT_C], F16, isOutput=False)
    w2l_r = nc.declare_dram_parameter("w2l_r", [128, OUT_C], F16, isOutput=False)
    b1r = nc.declare_dram_parameter("b1r", [128, 1], F32, isOutput=False)
    out = nc.declare_dram_parameter(
        "out", [N_BLK, 128, 2, 2 * NHALF], F32, isOutput=True
    )

    with tile.TileContext(nc) as tc:
        with (
            tc.tile_pool(name="consts", bufs=1) as cpool,
            tc.tile_pool(name="xa", bufs=3) as xa_pool,
            tc.tile_pool(name="xb", bufs=3) as xb_pool,
            tc.tile_pool(name="hsp", bufs=4) as hsp_pool,
            tc.tile_pool(name="ob", bufs=3) as ob_pool,
            tc.tile_pool(name="ph", bufs=4, space="PSUM") as ph_pool,
            tc.tile_pool(name="po", bufs=4, space="PSUM") as po_pool,
        ):
            w1ah_t = cpool.tile([128, HID], F16)
            nc.sync.dma_start(w1ah_t[:], w1a_h[:])
            w1al_t = cpool.tile([128, HID], F16)
            nc.sync.dma_start(w1al_t[:], w1a_l[:])
            wes_t = cpool.tile([96, HID], F16)
            nc.sync.dma_start(wes_t[:], w_es[:])
            w2h_t = cpool.tile([128, OUT_C], F16)
            nc.sync.dma_start(w2h_t[:], w2h_r[:])
            w2l_t = cpool.tile([128, OUT_C], F16)
            nc.sync.dma_start(w2l_t[:], w2l_r[:])
            b1r_t = cpool.tile([128, 1], F32)
            nc.sync.dma_start(b1r_t[:], b1r[:])

            warm_t = cpool.tile([128, NHALF], F16)
            nc.vector.memset(warm_t[:], 0.0)
            warm_ps = ph_pool.tile([128, NHALF], F32, tag="ph_t", name="warm_ps")
            for _ in range(56):
                nc.tensor.matmul(
                    warm_ps[:, :], warm_t[:, 0:128], warm_t[:, :],
                    start=True, stop=True,
                )

            for blk in range(N_BLK):
                xa_t = xa_pool.tile([128, 2, BLK_EDGES], F16)
                nc.sync.dma_start(xa_t[:], xta[blk])
                xb_t = xb_pool.tile([128, Q_PER_BLK, NHALF], F16, name="xb_t")
                nc.sync.dma_start(xb_t[0:96, :, :], xtb[blk])
                ob_t = ob_pool.tile([128, 2, 2 * NHALF], F32)

                for pair in range(Q_PER_BLK // 2):
                    hh_t = hsp_pool.tile([128, NHALF], F16, tag="hh", name="hh_t")
                    hl_t = hsp_pool.tile([128, NHALF], F16, tag="hl", name="hl_t")
                    t32_t = hsp_pool.tile([128, NHALF], F32, tag="t32", name="t32_t")
                    ph = [None, None]
                    po = [None, None]
                    # layer 1 for both tiles of the pair
                    for par in range(2):
                        q = 2 * pair + par
                        c0 = 64 * par
                        ph_t = ph_pool.tile([128, NHALF], F32, name="ph_t")
                        ph[par] = ph_t
                        xah = xa_t[:, 0, bass.ts(q, NHALF)]
                        xal = xa_t[:, 1, bass.ts(q, NHALF)]
                        nc.tensor.matmul(
                            ph_t[c0 : c0 + 64, :], w1ah_t[:, :], xah,
                            start=True, stop=False, tile_position=(0, c0),
                        )
                        nc.tensor.matmul(
                            ph_t[c0 : c0 + 64, :], w1al_t[:, :], xah,
                            start=False, stop=False, tile_position=(0, c0),
                        )
                        nc.tensor.matmul(
                            ph_t[c0 : c0 + 64, :], w1ah_t[:, :], xal,
                            start=False, stop=False, tile_position=(0, c0),
                        )
                        nc.tensor.matmul(
                            ph_t[c0 : c0 + 64, :], wes_t[:, :], xb_t[0:96, q, :],
                            start=False, stop=True, tile_position=(0, c0),
                        )
                    # h split ops for both tiles
                    for par in range(2):
                        c0 = 64 * par
                        ph_t = ph[par]
                        nc.vector.tensor_scalar(
                            hh_t[c0 : c0 + 64, :],
                            ph_t[c0 : c0 + 64, :],
                            b1r_t[c0 : c0 + 64, :],
                            0.0,
                            mybir.AluOpType.add,
                            mybir.AluOpType.max,
                        )
                        nc.scalar.activation(
                            t32_t[c0 : c0 + 64, :], ph_t[c0 : c0 + 64, :],
                            mybir.ActivationFunctionType.Relu,
                            bias=b1r_t[c0 : c0 + 64, :],
                        )
                        nc.vector.tensor_tensor(
                            hl_t[c0 : c0 + 64, :],
                            t32_t[c0 : c0 + 64, :],
                            hh_t[c0 : c0 + 64, :],
                            mybir.AluOpType.subtract,
                        )
                        po[par] = po_pool.tile([128, NHALF], F32, name="po_t")
                    # layer 2 interleaved: the two tiles use disjoint PE
                    # row AND column groups, so adjacent matmuls co-execute
                    for w_t, rhs_t, st, sp in (
                        (w2h_t, hh_t, True, False),
                        (w2l_t, hh_t, False, False),
                        (w2h_t, hl_t, False, True),
                    ):
                        for par in range(2):
                            c0 = 64 * par
                            nc.tensor.matmul(
                                po[par][c0 : c0 + 64, :],
                                w_t[c0 : c0 + 64, :],
                                rhs_t[c0 : c0 + 64, :],
                                start=st, stop=sp, tile_position=(c0, c0),
                            )
                    # PSUM -> SBUF copies (b2 added on host)
                    grp, cg = divmod(pair, 2)
                    for par in range(2):
                        c0 = 64 * par
                        nc.scalar.activation(
                            ob_t[c0 : c0 + 64, grp, bass.ts(cg, NHALF)],
                            po[par][c0 : c0 + 64, :],
                            mybir.ActivationFunctionType.Copy,
                        )
                nc.sync.dma_start(out[blk], ob_t[:])

    nc.compile()
    return nc


def _get_program():
    if "prog" not in _PROGRAM_CACHE:
        _PROGRAM_CACHE["prog"] = _build_program()
    return _PROGRAM_CACHE["prog"]


def _pad_rows(a, n):
    if a.shape[0] == n:
        return a
    pad = np.zeros((n - a.shape[0],) + a.shape[1:], dtype=a.dtype)
    return np.concatenate([a, pad], axis=0)


def _split16(a):
    """fp32 array -> (half hi, half lo) with hi + lo ~= a."""
    hi = a.astype(_NP_HALF)
    lo = (a - hi.astype(np.float32)).astype(_NP_HALF)
    return hi, lo


def _host_pack(v_i, v_j, e_ij, W1, b1, W2, b2):
    """Build per-core input maps in the device layouts."""
    v_i = np.ascontiguousarray(v_i, dtype=np.float32)
    v_j = np.ascontiguousarray(v_j, dtype=np.float32)
    e_ij = np.ascontiguousarray(e_ij, dtype=np.float32)

    Wx = np.asarray(W1[:128], dtype=np.float32)
    We = np.asarray(W1[128:160], dtype=np.float32)
    Wxh, Wxl = _split16(Wx)
    Weh, Wel = _split16(We)
    W2h, W2l = _split16(np.asarray(W2, dtype=np.float32))

    es_w = np.concatenate([Weh, Wel, Weh], axis=0)  # [96, 64] halfword

    weights = {
        "w1a_h": np.ascontiguousarray(Wxh),
        "w1a_l": np.ascontiguousarray(Wxl),
        "w_es": np.ascontiguousarray(es_w),
        "w2h_r": np.ascontiguousarray(np.tile(W2h, (2, 1))),
        "w2l_r": np.ascontiguousarray(np.tile(W2l, (2, 1))),
        "b1r": np.ascontiguousarray(np.tile(b1, 2)[:, None], dtype=np.float32),
    }

    in_maps = []
    for c in range(N_CORES):
        sl = slice(c * EPC, (c + 1) * EPC)
        vi = _pad_rows(v_i[sl], EPAD)    # [EPAD, 64]
        vj = _pad_rows(v_j[sl], EPAD)
        ec = _pad_rows(e_ij[sl], EPAD)   # [EPAD, 32]

        # xta[b, p, h, n] = (Ah|Al)[p, b*4096 + n],  A = [v_i^T; v_j^T]
        A = np.concatenate([vi.T, vj.T], axis=0)          # [128, EPAD] f32
        Ah, Al = _split16(A)
        st = np.stack([Ah, Al], axis=1)                   # [128, 2, EPAD]
        xta = np.ascontiguousarray(
            st.reshape(128, 2, N_BLK, BLK_EDGES).transpose(2, 0, 1, 3)
        )  # [N_BLK, 128, 2, 4096] half

        # e-stack [eh; eh; el] along K at rows 0:96 for every q
        eh, el = _split16(ec)                             # [EPAD, 32] each
        EST = np.concatenate([eh, eh, el], axis=1).T      # [96, EPAD] f16
        Tr = EST.reshape(96, N_BLK, Q_PER_BLK, NHALF)     # [r, b, q, n]
        xtb = np.ascontiguousarray(Tr.transpose(1, 0, 2, 3))

        in_maps.append({"xta": xta, "xtb": xtb, **weights})
    return in_maps


def _host_unpack(results, b2):
    """results: list of per-core dicts with 'out' [N_BLK, 128, 2, 1024]."""
    b2 = np.asarray(b2, dtype=np.float32)
    outs = []
    for c in range(N_CORES):
        o = np.asarray(results[c]["out"])
        # out[b, 64*par + p, grp, 512*cg + n]
        #   = OUT[b*4096 + grp*2048 + cg*1024 + par*512 + n, p]
        r = o.reshape(N_BLK, 2, 64, 2, 2, NHALF)   # [b, par, p, grp, cg, n]
        r = r.transpose(0, 3, 4, 1, 5, 2)           # [b, grp, cg, par, n, p]
        outs.append(np.ascontiguousarray(r).reshape(EPAD, OUT_C)[:EPC] + b2)
    return np.concatenate(outs, axis=0)


def kernel(v_i, v_j, e_ij, W1, b1, W2, b2):
    global LAST_RESULT
    nc = _get_program()
    in_maps = _host_pack(v_i, v_j, e_ij, W1, b1, W2, b2)
    res = run_bass_kernel_spmd(
        nc, in_maps, core_ids=list(range(N_CORES)), trace=_TRACE
    )
    LAST_RESULT = res
    return _host_unpack(res.results, b2)



# revision 6
# speedup vs baseline: 1.3071x; 1.3071x over previous
"""Trainium2 Bass kernel for the GNN message-update MLP:

    out = relu(concat([v_i, v_j, e_ij], -1) @ W1 + b1) @ W2 + b2

Strategy (memory-bound, E = 1M edges, data-parallel across 8 cores):
  - Shard edges across the 8 NeuronCores (125000 each, padded to 126976).
  - Pure fp16 I/O: activations ship as fp16 (half the HBM bytes of fp32)
    and the output is written back as fp16, converted to fp32 on host.
    PSUM accumulation stays fp32; end-to-end error ~5e-4 of scale.
  - Per 1024-edge pair (two 512-edge tiles on PSUM row halves via column
    tile_position): 2x K=128 x-matmuls + 2x K=32 e-matmuls (e rows are
    partition-stacked so each e-matmul streams from its own 32-row band)
    + ONE full-width layer-2 matmul with a block-diagonal [W2 0; 0 W2]
    stationary operand. 5 matmuls / 1024 edges, all N=512.
  - One [128,512] VectorE relu+bias (fp32 PSUM -> fp16) and one
    [128,512] ScalarE copy (PSUM -> fp16 SBUF) per pair - all
    element-wise work runs on full 128 partitions.
  - Layer-2 + output copy are software-pipelined one pair behind
    layer-1 so the PE queue never stalls on the vector engine.
  - Inputs stream on the sync-engine HWDGE queue, outputs on the
    scalar-engine HWDGE queue (independent FIFOs).
"""

import numpy as np

import concourse.bacc as bacc
import concourse.bass as bass
import concourse.mybir as mybir
import concourse.tile as tile
from concourse.bass_utils import run_bass_kernel_spmd

# ---- problem constants (hardcoded per harness contract) ----
E_TOTAL = 1_000_000
N_CORES = 8
IN_C = 64
IN_E = 32
HID = 64
OUT_C = 64

NHALF = 512                    # edges per 64-col output tile / matmul N
Q_PER_BLK = 8                  # 512-edge tiles per block
P_PER_BLK = Q_PER_BLK // 2     # 4 pairs per block
BLK_EDGES = NHALF * Q_PER_BLK  # 4096
EPC = E_TOTAL // N_CORES       # 125000 edges per core
N_BLK = -(-EPC // BLK_EDGES)   # 31
EPAD = N_BLK * BLK_EDGES       # 126976

XCOLS = BLK_EDGES              # 4096 x-columns per block
ECOLS = BLK_EDGES // 4         # 1024 e-columns per block (32-row bands)
INCOLS = XCOLS + ECOLS         # 5120

F32 = mybir.dt.float32
F16 = mybir.dt.float16

# test.py hooks
_TRACE = False
LAST_RESULT = None

_PROGRAM_CACHE = {}


def _build_program():
    nc = bacc.Bacc(
        "TRN2",
        target_bir_lowering=False,
        debug=False,
        num_devices=N_CORES,
    )

    xin = nc.declare_dram_parameter(
        "xin", [N_BLK, 128, INCOLS], F16, isOutput=False
    )
    wx = nc.declare_dram_parameter("wx", [128, HID], F16, isOutput=False)
    wes4 = nc.declare_dram_parameter("wes4", [128, HID], F16, isOutput=False)
    w2d = nc.declare_dram_parameter("w2d", [128, 128], F16, isOutput=False)
    b1r = nc.declare_dram_parameter("b1r", [128, 1], F32, isOutput=False)
    out = nc.declare_dram_parameter(
        "out", [N_BLK, 128, P_PER_BLK * NHALF], F16, isOutput=True
    )

    with tile.TileContext(nc) as tc:
        with (
            tc.tile_pool(name="consts", bufs=1) as cpool,
            tc.tile_pool(name="xi", bufs=3) as xi_pool,
            tc.tile_pool(name="hh", bufs=4) as hh_pool,
            tc.tile_pool(name="ob", bufs=3) as ob_pool,
            tc.tile_pool(name="ph", bufs=4, space="PSUM") as ph_pool,
            tc.tile_pool(name="po", bufs=3, space="PSUM") as po_pool,
        ):
            wx_t = cpool.tile([128, HID], F16)
            nc.sync.dma_start(wx_t[:], wx[:])
            wes4_t = cpool.tile([128, HID], F16)
            nc.sync.dma_start(wes4_t[:], wes4[:])
            w2d_t = cpool.tile([128, 128], F16)
            nc.sync.dma_start(w2d_t[:], w2d[:])
            b1r_t = cpool.tile([128, 1], F32)
            nc.sync.dma_start(b1r_t[:], b1r[:])

            # warm the PE clock gate (HAM) while the first input DMA is in
            # flight: ~4us of matmul activity raises the PE from 1.2 to
            # 2.4 GHz before the real work starts.
            warm_t = cpool.tile([128, NHALF], F16)
            nc.vector.memset(warm_t[:], 0.0)
            warm_ps = ph_pool.tile([128, NHALF], F32, tag="ph_t", name="warm_ps")
            for _ in range(14):
                nc.tensor.matmul(
                    warm_ps[:, :], warm_t[:, 0:128], warm_t[:, :],
                    start=True, stop=True,
                )

            # software pipeline state: (hh tile, ob tile, pair idx, blk)
            pending = None

            def emit_l2(p):
                hh, ob_t, pr, b = p
                po = po_pool.tile([128, NHALF], F32, tag="po_t", name="po")
                nc.tensor.matmul(
                    po[:, :], w2d_t[:, :], hh[:, :],
                    start=True, stop=True, tile_position=(0, 0),
                )
                nc.scalar.activation(
                    ob_t[:, pr * NHALF : (pr + 1) * NHALF], po[:, :],
                    mybir.ActivationFunctionType.Copy,
                )
                if pr == P_PER_BLK - 1:
                    nc.scalar.dma_start(out[b], ob_t[:])

            for blk in range(N_BLK):
                xi_t = xi_pool.tile([128, INCOLS], F16)
                nc.sync.dma_start(xi_t[:], xin[blk])
                ob_t = ob_pool.tile([128, P_PER_BLK * NHALF], F16)

                for pr in range(P_PER_BLK):
                    # tiles qa = 2*pr, qb = 2*pr+1; e-band rows: 32*(q%4),
                    # e-cols: XCOLS + 512*(q//4)
                    qa, qb = 2 * pr, 2 * pr + 1
                    ra, rb = 32 * (qa % 4), 32 * (qb % 4)
                    ea = XCOLS + NHALF * (qa // 4)
                    eb_ = XCOLS + NHALF * (qb // 4)
                    ph = ph_pool.tile([128, NHALF], F32, tag="ph_t", name="ph")
                    # layer 1, tile qa -> PSUM rows 0:64
                    nc.tensor.matmul(
                        ph[0:64, :], wx_t[:, :],
                        xi_t[:, qa * NHALF : (qa + 1) * NHALF],
                        start=True, stop=False, tile_position=(0, 0),
                    )
                    nc.tensor.matmul(
                        ph[0:64, :],
                        wes4_t[ra : ra + 32, :],
                        xi_t[ra : ra + 32, ea : ea + NHALF],
                        start=False, stop=True, tile_position=(ra, 0),
                    )
                    # layer 1, tile qb -> PSUM rows 64:128
                    nc.tensor.matmul(
                        ph[64:128, :], wx_t[:, :],
                        xi_t[:, qb * NHALF : (qb + 1) * NHALF],
                        start=True, stop=False, tile_position=(0, 64),
                    )
                    nc.tensor.matmul(
                        ph[64:128, :],
                        wes4_t[rb : rb + 32, :],
                        xi_t[rb : rb + 32, eb_ : eb_ + NHALF],
                        start=False, stop=True, tile_position=(rb, 64),
                    )
                    # relu(ph + b1) -> fp16, full 128 partitions
                    hh = hh_pool.tile([128, NHALF], F16, tag="hh", name="hh")
                    nc.vector.tensor_scalar(
                        hh[:, :], ph[:, :], b1r_t[:, :], 0.0,
                        mybir.AluOpType.add, mybir.AluOpType.max,
                    )
                    # layer 2 of the PREVIOUS pair (software pipelining):
                    # its hh is ready, so the PE queue never waits on DVE.
                    if pending is not None:
                        emit_l2(pending)
                    pending = (hh, ob_t, pr, blk)

            if pending is not None:
                emit_l2(pending)

    nc.compile()
    return nc


def _get_program():
    if "prog" not in _PROGRAM_CACHE:
        _PROGRAM_CACHE["prog"] = _build_program()
    return _PROGRAM_CACHE["prog"]


def _pad_rows(a, n):
    if a.shape[0] == n:
        return a
    pad = np.zeros((n - a.shape[0],) + a.shape[1:], dtype=a.dtype)
    return np.concatenate([a, pad], axis=0)


def _host_pack(v_i, v_j, e_ij, W1, b1, W2, b2):
    """Build per-core input maps in the device layouts."""
    W1 = np.asarray(W1, dtype=np.float32)
    W2 = np.asarray(W2, dtype=np.float32)
    wx_h = W1[:128].astype(np.float16)
    wes_h = W1[128:160].astype(np.float16)
    w2_h = W2.astype(np.float16)

    w2d = np.zeros((128, 128), dtype=np.float16)
    w2d[0:64, 0:64] = w2_h
    w2d[64:128, 64:128] = w2_h

    weights = {
        "wx": np.ascontiguousarray(wx_h),
        "wes4": np.ascontiguousarray(np.tile(wes_h, (4, 1))),
        "w2d": w2d,
        "b1r": np.ascontiguousarray(np.tile(b1, 2)[:, None], dtype=np.float32),
    }

    in_maps = []
    for c in range(N_CORES):
        sl = slice(c * EPC, (c + 1) * EPC)
        vi = _pad_rows(np.asarray(v_i[sl], dtype=np.float16), EPAD)
        vj = _pad_rows(np.asarray(v_j[sl], dtype=np.float16), EPAD)
        ec = _pad_rows(np.asarray(e_ij[sl], dtype=np.float16), EPAD)

        # x-part: [vi^T; vj^T] -> [N_BLK, 128, 4096]
        X = np.concatenate([vi.T, vj.T], axis=0)          # [128, EPAD] f16
        xa = X.reshape(128, N_BLK, XCOLS).transpose(1, 0, 2)

        # e-part: tile q = 4h + i -> rows 32i:32i+32, cols 512h:512h+512
        ET = ec.T                                          # [32, EPAD] f16
        ebd = ET.reshape(32, N_BLK, 2, 4, NHALF).transpose(1, 3, 0, 2, 4)
        ebd = ebd.reshape(N_BLK, 128, ECOLS)               # [blk, 32i+r, 512h+n]

        xi_full = np.concatenate([xa, ebd], axis=2)        # [N_BLK, 128, 5120]
        in_maps.append({"xin": np.ascontiguousarray(xi_full), **weights})
    return in_maps


def _host_unpack(results, b2):
    """results: per-core dicts with 'out' [N_BLK, 128, 2048] f16."""
    b2 = np.asarray(b2, dtype=np.float32)
    outs = []
    for c in range(N_CORES):
        o = np.asarray(results[c]["out"])
        # o[blk, 64r + j, 512p + n] = OUT[blk*4096 + (2p + r)*512 + n, j]
        r = o.reshape(N_BLK, 2, 64, P_PER_BLK, NHALF)  # [blk, r, j, p, n]
        r = r.transpose(0, 3, 1, 4, 2)                  # [blk, p, r, n, j]
        r = np.ascontiguousarray(r).reshape(EPAD, OUT_C)[:EPC]
        outs.append(r.astype(np.float32) + b2)
    return np.concatenate(outs, axis=0)


def kernel(v_i, v_j, e_ij, W1, b1, W2, b2):
    global LAST_RESULT
    nc = _get_program()
    in_maps = _host_pack(v_i, v_j, e_ij, W1, b1, W2, b2)
    res = run_bass_kernel_spmd(
        nc, in_maps, core_ids=list(range(N_CORES)), trace=_TRACE
    )
    LAST_RESULT = res
    return _host_unpack(res.results, b2)


# revision 10
# speedup vs baseline: 1.6517x; 1.2636x over previous
"""Trainium2 Bass kernel for the GNN message-update MLP:

    out = relu(concat([v_i, v_j, e_ij], -1) @ W1 + b1) @ W2 + b2

Strategy (memory-bound, E = 1M edges, data-parallel across 8 cores):
  - Shard edges across the 8 NeuronCores (125000 each, padded to 126976).
  - Pure fp16 I/O: activations ship as fp16 (half the HBM bytes of fp32)
    and the output is written back as fp16, converted to fp32 on host.
    PSUM accumulation stays fp32; end-to-end error ~5e-4 of scale.
  - Per 1024-edge pair (two 512-edge tiles on PSUM row halves via column
    tile_position): 2x K=128 x-matmuls + 2x K=32 e-matmuls (e rows are
    partition-stacked so each e-matmul streams from its own 32-row band)
    + ONE full-width layer-2 matmul with a block-diagonal [W2 0; 0 W2]
    stationary operand. 5 matmuls / 1024 edges, all N=512.
  - One [128,512] VectorE relu+bias (fp32 PSUM -> fp16) and one
    [128,512] ScalarE copy (PSUM -> fp16 SBUF) per pair - all
    element-wise work runs on full 128 partitions.
  - Layer-2 + output copy are software-pipelined one pair behind
    layer-1 so the PE queue never stalls on the vector engine.
  - Inputs stream on the sync-engine HWDGE queue, outputs on the
    scalar-engine HWDGE queue (independent FIFOs).
"""

import numpy as np

import concourse.bacc as bacc
import concourse.bass as bass
import concourse.mybir as mybir
import concourse.tile as tile
from concourse.bass_utils import run_bass_kernel_spmd

# ---- problem constants (hardcoded per harness contract) ----
E_TOTAL = 1_000_000
N_CORES = 8
IN_C = 64
IN_E = 32
HID = 64
OUT_C = 64

NHALF = 512                    # edges per 64-col output tile / matmul N
Q_PER_BLK = 8                  # 512-edge tiles per block
P_PER_BLK = Q_PER_BLK // 2     # 4 pairs per block
BLK_EDGES = NHALF * Q_PER_BLK  # 4096
EPC = E_TOTAL // N_CORES       # 125000 edges per core
N_BLK = -(-EPC // BLK_EDGES)   # 31
EPAD = N_BLK * BLK_EDGES       # 126976

XCOLS = BLK_EDGES              # 4096 x-columns per block
ECOLS = BLK_EDGES // 4         # 1024 e-columns per block (32-row bands)
INCOLS = XCOLS + ECOLS         # 5120

F32 = mybir.dt.float32
F16 = mybir.dt.float16

# test.py hooks
_TRACE = False
LAST_RESULT = None

_PROGRAM_CACHE = {}


def _build_program():
    nc = bacc.Bacc(
        "TRN2",
        target_bir_lowering=False,
        debug=False,
        num_devices=N_CORES,
    )

    xin = nc.declare_dram_parameter(
        "xin", [N_BLK, 128, INCOLS], F16, isOutput=False
    )
    wx = nc.declare_dram_parameter("wx", [128, HID], F16, isOutput=False)
    wes2d = nc.declare_dram_parameter("wes2d", [128, 128], F16, isOutput=False)
    w2d = nc.declare_dram_parameter("w2d", [128, 128], F16, isOutput=False)
    b1r = nc.declare_dram_parameter("b1r", [128, 1], F32, isOutput=False)
    out = nc.declare_dram_parameter(
        "out", [N_BLK, 128, P_PER_BLK * NHALF], F16, isOutput=True
    )

    with tile.TileContext(nc) as tc:
        with (
            tc.tile_pool(name="consts", bufs=1) as cpool,
            tc.tile_pool(name="xi", bufs=3) as xi_pool,
            tc.tile_pool(name="hh", bufs=4) as hh_pool,
            tc.tile_pool(name="ob", bufs=3) as ob_pool,
            tc.tile_pool(name="ph", bufs=4, space="PSUM") as ph_pool,
            tc.tile_pool(name="po", bufs=3, space="PSUM") as po_pool,
        ):
            wx_t = cpool.tile([128, HID], F16)
            nc.sync.dma_start(wx_t[:], wx[:])
            wes2d_t = cpool.tile([128, 128], F16)
            nc.sync.dma_start(wes2d_t[:], wes2d[:])
            w2d_t = cpool.tile([128, 128], F16)
            nc.sync.dma_start(w2d_t[:], w2d[:])
            b1r_t = cpool.tile([128, 1], F32)
            nc.sync.dma_start(b1r_t[:], b1r[:])

            # warm the PE clock gate (HAM) while the first input DMA is in
            # flight: matmul activity raises the PE from 1.2 to 2.4 GHz.
            # Sized so warmup ends right when block 0's first chunk lands:
            # ANY >~2us PE idle gap drops the clock and it never re-raises
            # under a saturated (gap-free) matmul stream.
            warm_t = cpool.tile([128, NHALF], F16)
            nc.vector.memset(warm_t[:], 0.0)
            warm_ps = ph_pool.tile([128, NHALF], F32, tag="ph_t", name="warm_ps")
            for _ in range(12):
                nc.tensor.matmul(
                    warm_ps[:, :], warm_t[:, 0:128], warm_t[:, :],
                    start=True, stop=True,
                )

            # software pipeline state: (hh tile, ob tile, pair idx, blk)
            pending = None

            def emit_l2(p):
                hh, ob_t, pr, b = p
                po = po_pool.tile([128, NHALF], F32, tag="po_t", name="po")
                nc.tensor.matmul(
                    po[:, :], w2d_t[:, :], hh[:, :],
                    start=True, stop=True, tile_position=(0, 0),
                )
                nc.scalar.activation(
                    ob_t[:, pr * NHALF : (pr + 1) * NHALF], po[:, :],
                    mybir.ActivationFunctionType.Copy,
                )
                if pr == P_PER_BLK - 1:
                    nc.scalar.dma_start(out[b], ob_t[:])

            for blk in range(N_BLK):
                xi_t = xi_pool.tile([128, INCOLS], F16)
                if blk == 0:
                    # chunked first-block DMA: e-columns first, then the
                    # four 1024-col x chunks, so pair 0 becomes runnable
                    # ~1.6us after the first bytes land and the PE hands
                    # off from warmup to real matmuls with no idle gap.
                    nc.sync.dma_start(
                        xi_t[:, XCOLS:INCOLS], xin[blk, :, XCOLS:INCOLS]
                    )
                    for ck in range(P_PER_BLK):
                        c0 = ck * 2 * NHALF
                        nc.sync.dma_start(
                            xi_t[:, c0 : c0 + 2 * NHALF],
                            xin[blk, :, c0 : c0 + 2 * NHALF],
                        )
                else:
                    nc.sync.dma_start(xi_t[:], xin[blk])
                ob_t = ob_pool.tile([128, P_PER_BLK * NHALF], F16)

                for pr in range(P_PER_BLK):
                    # tiles qa = 2*pr, qb = 2*pr+1 -> PSUM rows 0:64 /
                    # 64:128; both e-tiles sit stacked in one 64-row band
                    # (rows 64*(pr%2)..+64, cols XCOLS + 512*(pr//2)), so
                    # ONE K=64 full-width matmul with blockdiag(We, We)
                    # adds both e contributions.
                    qa, qb = 2 * pr, 2 * pr + 1
                    er = 64 * (pr % 2)
                    ec = XCOLS + NHALF * (pr // 2)
                    ph = ph_pool.tile([128, NHALF], F32, tag="ph_t", name="ph")
                    nc.tensor.matmul(
                        ph[0:64, :], wx_t[:, :],
                        xi_t[:, qa * NHALF : (qa + 1) * NHALF],
                        start=True, stop=False, tile_position=(0, 0),
                    )
                    nc.tensor.matmul(
                        ph[64:128, :], wx_t[:, :],
                        xi_t[:, qb * NHALF : (qb + 1) * NHALF],
                        start=True, stop=False, tile_position=(0, 64),
                    )
                    nc.tensor.matmul(
                        ph[:, :],
                        wes2d_t[er : er + 64, :],
                        xi_t[er : er + 64, ec : ec + NHALF],
                        start=False, stop=True, tile_position=(er, 0),
                        skip_group_check=True,
                    )
                    # relu(ph + b1) -> fp16, full 128 partitions
                    hh = hh_pool.tile([128, NHALF], F16, tag="hh", name="hh")
                    nc.vector.tensor_scalar(
                        hh[:, :], ph[:, :], b1r_t[:, :], 0.0,
                        mybir.AluOpType.add, mybir.AluOpType.max,
                    )
                    # layer 2 of the PREVIOUS pair (software pipelining):
                    # its hh is ready, so the PE queue never waits on DVE.
                    if pending is not None:
                        emit_l2(pending)
                    pending = (hh, ob_t, pr, blk)

            if pending is not None:
                emit_l2(pending)

    nc.compile()
    return nc


def _get_program():
    if "prog" not in _PROGRAM_CACHE:
        _PROGRAM_CACHE["prog"] = _build_program()
    return _PROGRAM_CACHE["prog"]


def _pad_rows(a, n):
    if a.shape[0] == n:
        return a
    pad = np.zeros((n - a.shape[0],) + a.shape[1:], dtype=a.dtype)
    return np.concatenate([a, pad], axis=0)


def _host_pack(v_i, v_j, e_ij, W1, b1, W2, b2):
    """Build per-core input maps in the device layouts."""
    W1 = np.asarray(W1, dtype=np.float32)
    W2 = np.asarray(W2, dtype=np.float32)
    wx_h = W1[:128].astype(np.float16)
    wes_h = W1[128:160].astype(np.float16)
    w2_h = W2.astype(np.float16)

    w2d = np.zeros((128, 128), dtype=np.float16)
    w2d[0:64, 0:64] = w2_h
    w2d[64:128, 64:128] = w2_h

    # blockdiag(We, We) [64, 128], tiled twice down the partitions so the
    # e-matmul's stationary operand sits at the same base partition as its
    # moving band (rows 0:64 or 64:128).
    wes2d_half = np.zeros((64, 128), dtype=np.float16)
    wes2d_half[0:32, 0:64] = wes_h
    wes2d_half[32:64, 64:128] = wes_h
    wes2d = np.tile(wes2d_half, (2, 1))

    weights = {
        "wx": np.ascontiguousarray(wx_h),
        "wes2d": np.ascontiguousarray(wes2d),
        "w2d": w2d,
        "b1r": np.ascontiguousarray(np.tile(b1, 2)[:, None], dtype=np.float32),
    }

    in_maps = []
    for c in range(N_CORES):
        sl = slice(c * EPC, (c + 1) * EPC)
        vi = _pad_rows(np.asarray(v_i[sl], dtype=np.float16), EPAD)
        vj = _pad_rows(np.asarray(v_j[sl], dtype=np.float16), EPAD)
        ec = _pad_rows(np.asarray(e_ij[sl], dtype=np.float16), EPAD)

        # x-part: [vi^T; vj^T] -> [N_BLK, 128, 4096]
        X = np.concatenate([vi.T, vj.T], axis=0)          # [128, EPAD] f16
        xa = X.reshape(128, N_BLK, XCOLS).transpose(1, 0, 2)

        # e-part: tile q = 4h + i -> rows 32i:32i+32, cols 512h:512h+512
        ET = ec.T                                          # [32, EPAD] f16
        ebd = ET.reshape(32, N_BLK, 2, 4, NHALF).transpose(1, 3, 0, 2, 4)
        ebd = ebd.reshape(N_BLK, 128, ECOLS)               # [blk, 32i+r, 512h+n]

        xi_full = np.concatenate([xa, ebd], axis=2)        # [N_BLK, 128, 5120]
        in_maps.append({"xin": np.ascontiguousarray(xi_full), **weights})
    return in_maps


def _host_unpack(results, b2):
    """results: per-core dicts with 'out' [N_BLK, 128, 2048] f16."""
    b2 = np.asarray(b2, dtype=np.float32)
    outs = []
    for c in range(N_CORES):
        o = np.asarray(results[c]["out"])
        # o[blk, 64r + j, 512p + n] = OUT[blk*4096 + (2p + r)*512 + n, j]
        r = o.reshape(N_BLK, 2, 64, P_PER_BLK, NHALF)  # [blk, r, j, p, n]
        r = r.transpose(0, 3, 1, 4, 2)                  # [blk, p, r, n, j]
        r = np.ascontiguousarray(r).reshape(EPAD, OUT_C)[:EPC]
        outs.append(r.astype(np.float32) + b2)
    return np.concatenate(outs, axis=0)


def kernel(v_i, v_j, e_ij, W1, b1, W2, b2):
    global LAST_RESULT
    nc = _get_program()
    in_maps = _host_pack(v_i, v_j, e_ij, W1, b1, W2, b2)
    res = run_bass_kernel_spmd(
        nc, in_maps, core_ids=list(range(N_CORES)), trace=_TRACE
    )
    LAST_RESULT = res
    return _host_unpack(res.results, b2)


# revision 13
# speedup vs baseline: 2.1490x; 1.3011x over previous
"""Trainium2 Bass kernel for the GNN message-update MLP:

    out = relu(concat([v_i, v_j, e_ij], -1) @ W1 + b1) @ W2 + b2

Strategy (memory-bound, E = 1M edges, data-parallel across 8 cores):
  - Shard edges across the 8 NeuronCores (125000 each, padded to 126976).
  - Pure fp16 I/O: activations ship as fp16 (half the HBM bytes of fp32)
    and the output is written back as fp16, converted to fp32 on host.
    PSUM accumulation stays fp32; end-to-end error ~5e-4 of scale.
  - Per 1024-edge pair (two 512-edge tiles on PSUM row halves via column
    tile_position): 2x K=128 x-matmuls + 2x K=32 e-matmuls (e rows are
    partition-stacked so each e-matmul streams from its own 32-row band)
    + ONE full-width layer-2 matmul with a block-diagonal [W2 0; 0 W2]
    stationary operand. 5 matmuls / 1024 edges, all N=512.
  - One [128,512] VectorE relu+bias (fp32 PSUM -> fp16) and one
    [128,512] ScalarE copy (PSUM -> fp16 SBUF) per pair - all
    element-wise work runs on full 128 partitions.
  - Layer-2 + output copy are software-pipelined one pair behind
    layer-1 so the PE queue never stalls on the vector engine.
  - Inputs stream on the sync-engine HWDGE queue, outputs on the
    scalar-engine HWDGE queue (independent FIFOs).
"""

import numpy as np

import concourse.bacc as bacc
import concourse.bass as bass
import concourse.mybir as mybir
import concourse.tile as tile
from concourse.bass_utils import run_bass_kernel_spmd

# ---- problem constants (hardcoded per harness contract) ----
E_TOTAL = 1_000_000
N_CORES = 8
IN_C = 64
IN_E = 32
HID = 64
OUT_C = 64

NHALF = 512                    # edges per 64-col output tile / matmul N
Q_PER_BLK = 8                  # 512-edge tiles per block
P_PER_BLK = Q_PER_BLK // 2     # 4 pairs per block
BLK_EDGES = NHALF * Q_PER_BLK  # 4096
EPC = E_TOTAL // N_CORES       # 125000 edges per core
N_BLK = -(-EPC // BLK_EDGES)   # 31
EPAD = N_BLK * BLK_EDGES       # 126976

XCOLS = BLK_EDGES              # 4096 x-columns per block
ECOLS = BLK_EDGES // 4         # 1024 e-columns per block (32-row bands)
INCOLS = XCOLS + ECOLS         # 5120

F32 = mybir.dt.float32
F16 = mybir.dt.float16

# test.py hooks
_TRACE = False
LAST_RESULT = None

_PROGRAM_CACHE = {}


def _build_program():
    nc = bacc.Bacc(
        "TRN2",
        target_bir_lowering=False,
        debug=False,
        num_devices=N_CORES,
    )

    xin = nc.declare_dram_parameter(
        "xin", [N_BLK, 128, INCOLS], F16, isOutput=False
    )
    wx = nc.declare_dram_parameter("wx", [128, HID], F16, isOutput=False)
    wes2d = nc.declare_dram_parameter("wes2d", [128, 128], F16, isOutput=False)
    w2d = nc.declare_dram_parameter("w2d", [128, 128], F16, isOutput=False)
    b1r = nc.declare_dram_parameter("b1r", [128, 1], F32, isOutput=False)
    out = nc.declare_dram_parameter(
        "out", [N_BLK, 128, P_PER_BLK * NHALF], F16, isOutput=True
    )

    with tile.TileContext(nc) as tc:
        with (
            tc.tile_pool(name="consts", bufs=1) as cpool,
            tc.tile_pool(name="xi", bufs=4) as xi_pool,
            tc.tile_pool(name="hh", bufs=4) as hh_pool,
            tc.tile_pool(name="ob", bufs=3) as ob_pool,
            tc.tile_pool(name="ph", bufs=4, space="PSUM") as ph_pool,
            tc.tile_pool(name="po", bufs=3, space="PSUM") as po_pool,
        ):
            wx_t = cpool.tile([128, HID], F16)
            nc.sync.dma_start(wx_t[:], wx[:])
            wes2d_t = cpool.tile([128, 128], F16)
            nc.sync.dma_start(wes2d_t[:], wes2d[:])
            w2d_t = cpool.tile([128, 128], F16)
            nc.sync.dma_start(w2d_t[:], w2d[:])
            b1r_t = cpool.tile([128, 1], F32)
            nc.sync.dma_start(b1r_t[:], b1r[:])

            # No PE warmup: the HAM clock gate raises 1.2->2.4 GHz after
            # ~2 busy windows of the REAL matmul stream and sticks, as
            # long as the stream never stalls >~1us. A warmup that raises
            # the clock early just makes the first pairs outrun the
            # chunked first-block DMA, stall, and drop the clock for good
            # (a cold-saturated PE never re-raises).

            # software pipeline: layer-2 runs TWO pairs behind layer-1 so
            # the PE queue never waits on the vector engine's relu+sem
            # latency. entries: (hh tile, ob tile, pair idx, blk)
            pending = []

            def emit_l2(p):
                hh, ob_t, pr, b = p
                po = po_pool.tile([128, NHALF], F32, tag="po_t", name="po")
                nc.tensor.matmul(
                    po[:, :], w2d_t[:, :], hh[:, :],
                    start=True, stop=True, tile_position=(0, 0),
                )
                nc.scalar.activation(
                    ob_t[:, pr * NHALF : (pr + 1) * NHALF], po[:, :],
                    mybir.ActivationFunctionType.Copy,
                )
                if pr == P_PER_BLK - 1:
                    nc.scalar.dma_start(out[b], ob_t[:])

            for blk in range(N_BLK):
                xi_t = xi_pool.tile([128, INCOLS], F16)
                if blk == 0:
                    # chunked first-block DMA: e-columns first, then the
                    # four 1024-col x chunks, so pair 0 becomes runnable
                    # ~1.6us after the first bytes land and the PE hands
                    # off from warmup to real matmuls with no idle gap.
                    nc.sync.dma_start(
                        xi_t[:, XCOLS:INCOLS], xin[blk, :, XCOLS:INCOLS]
                    )
                    for ck in range(P_PER_BLK):
                        c0 = ck * 2 * NHALF
                        nc.sync.dma_start(
                            xi_t[:, c0 : c0 + 2 * NHALF],
                            xin[blk, :, c0 : c0 + 2 * NHALF],
                        )
                else:
                    nc.sync.dma_start(xi_t[:], xin[blk])
                ob_t = ob_pool.tile([128, P_PER_BLK * NHALF], F16)

                for pr in range(P_PER_BLK):
                    # tiles qa = 2*pr, qb = 2*pr+1 -> PSUM rows 0:64 /
                    # 64:128; both e-tiles sit stacked in one 64-row band
                    # (rows 64*(pr%2)..+64, cols XCOLS + 512*(pr//2)), so
                    # ONE K=64 full-width matmul with blockdiag(We, We)
                    # adds both e contributions.
                    qa, qb = 2 * pr, 2 * pr + 1
                    er = 64 * (pr % 2)
                    ec = XCOLS + NHALF * (pr // 2)
                    ph = ph_pool.tile([128, NHALF], F32, tag="ph_t", name="ph")
                    nc.tensor.matmul(
                        ph[0:64, :], wx_t[:, :],
                        xi_t[:, qa * NHALF : (qa + 1) * NHALF],
                        start=True, stop=False, tile_position=(0, 0),
                    )
                    nc.tensor.matmul(
                        ph[64:128, :], wx_t[:, :],
                        xi_t[:, qb * NHALF : (qb + 1) * NHALF],
                        start=True, stop=False, tile_position=(0, 64),
                    )
                    nc.tensor.matmul(
                        ph[:, :],
                        wes2d_t[er : er + 64, :],
                        xi_t[er : er + 64, ec : ec + NHALF],
                        start=False, stop=True, tile_position=(er, 0),
                        skip_group_check=True,
                    )
                    # relu(ph + b1) -> fp16, full 128 partitions
                    hh = hh_pool.tile([128, NHALF], F16, tag="hh", name="hh")
                    nc.vector.tensor_scalar(
                        hh[:, :], ph[:, :], b1r_t[:, :], 0.0,
                        mybir.AluOpType.add, mybir.AluOpType.max,
                    )
                    # layer 2 from two pairs ago (software pipelining)
                    if len(pending) == 2:
                        emit_l2(pending.pop(0))
                    pending.append((hh, ob_t, pr, blk))

            for p in pending:
                emit_l2(p)

    nc.compile()
    return nc


def _get_program():
    if "prog" not in _PROGRAM_CACHE:
        _PROGRAM_CACHE["prog"] = _build_program()
    return _PROGRAM_CACHE["prog"]


def _pad_rows(a, n):
    if a.shape[0] == n:
        return a
    pad = np.zeros((n - a.shape[0],) + a.shape[1:], dtype=a.dtype)
    return np.concatenate([a, pad], axis=0)


def _host_pack(v_i, v_j, e_ij, W1, b1, W2, b2):
    """Build per-core input maps in the device layouts."""
    W1 = np.asarray(W1, dtype=np.float32)
    W2 = np.asarray(W2, dtype=np.float32)
    wx_h = W1[:128].astype(np.float16)
    wes_h = W1[128:160].astype(np.float16)
    w2_h = W2.astype(np.float16)

    w2d = np.zeros((128, 128), dtype=np.float16)
    w2d[0:64, 0:64] = w2_h
    w2d[64:128, 64:128] = w2_h

    # blockdiag(We, We) [64, 128], tiled twice down the partitions so the
    # e-matmul's stationary operand sits at the same base partition as its
    # moving band (rows 0:64 or 64:128).
    wes2d_half = np.zeros((64, 128), dtype=np.float16)
    wes2d_half[0:32, 0:64] = wes_h
    wes2d_half[32:64, 64:128] = wes_h
    wes2d = np.tile(wes2d_half, (2, 1))

    weights = {
        "wx": np.ascontiguousarray(wx_h),
        "wes2d": np.ascontiguousarray(wes2d),
        "w2d": w2d,
        "b1r": np.ascontiguousarray(np.tile(b1, 2)[:, None], dtype=np.float32),
    }

    in_maps = []
    for c in range(N_CORES):
        sl = slice(c * EPC, (c + 1) * EPC)
        vi = _pad_rows(np.asarray(v_i[sl], dtype=np.float16), EPAD)
        vj = _pad_rows(np.asarray(v_j[sl], dtype=np.float16), EPAD)
        ec = _pad_rows(np.asarray(e_ij[sl], dtype=np.float16), EPAD)

        # x-part: [vi^T; vj^T] -> [N_BLK, 128, 4096]
        X = np.concatenate([vi.T, vj.T], axis=0)          # [128, EPAD] f16
        xa = X.reshape(128, N_BLK, XCOLS).transpose(1, 0, 2)

        # e-part: tile q = 4h + i -> rows 32i:32i+32, cols 512h:512h+512
        ET = ec.T                                          # [32, EPAD] f16
        ebd = ET.reshape(32, N_BLK, 2, 4, NHALF).transpose(1, 3, 0, 2, 4)
        ebd = ebd.reshape(N_BLK, 128, ECOLS)               # [blk, 32i+r, 512h+n]

        xi_full = np.concatenate([xa, ebd], axis=2)        # [N_BLK, 128, 5120]
        in_maps.append({"xin": np.ascontiguousarray(xi_full), **weights})
    return in_maps


def _host_unpack(results, b2):
    """results: per-core dicts with 'out' [N_BLK, 128, 2048] f16."""
    b2 = np.asarray(b2, dtype=np.float32)
    outs = []
    for c in range(N_CORES):
        o = np.asarray(results[c]["out"])
        # o[blk, 64r + j, 512p + n] = OUT[blk*4096 + (2p + r)*512 + n, j]
        r = o.reshape(N_BLK, 2, 64, P_PER_BLK, NHALF)  # [blk, r, j, p, n]
        r = r.transpose(0, 3, 1, 4, 2)                  # [blk, p, r, n, j]
        r = np.ascontiguousarray(r).reshape(EPAD, OUT_C)[:EPC]
        outs.append(r.astype(np.float32) + b2)
    return np.concatenate(outs, axis=0)


def kernel(v_i, v_j, e_ij, W1, b1, W2, b2):
    global LAST_RESULT
    nc = _get_program()
    in_maps = _host_pack(v_i, v_j, e_ij, W1, b1, W2, b2)
    res = run_bass_kernel_spmd(
        nc, in_maps, core_ids=list(range(N_CORES)), trace=_TRACE
    )
    LAST_RESULT = res
    return _host_unpack(res.results, b2)


# revision 19
# speedup vs baseline: 2.7357x; 1.2730x over previous
"""Trainium2 Bass kernel for the GNN message-update MLP:

    out = relu(concat([v_i, v_j, e_ij], -1) @ W1 + b1) @ W2 + b2

Strategy (memory-bound, E = 1M edges, data-parallel across 8 cores):
  - Shard edges across the 8 NeuronCores (125000 each, padded to 126976).
  - Pure fp16 I/O: activations ship as fp16 (half the HBM bytes of fp32)
    and the output is written back as fp16, converted to fp32 on host.
    PSUM accumulation stays fp32; end-to-end error ~5e-4 of scale.
  - Per 1024-edge pair (two 512-edge tiles on PSUM row halves via column
    tile_position): 2x K=128 x-matmuls + 2x K=32 e-matmuls (e rows are
    partition-stacked so each e-matmul streams from its own 32-row band)
    + ONE full-width layer-2 matmul with a block-diagonal [W2 0; 0 W2]
    stationary operand. 5 matmuls / 1024 edges, all N=512.
  - One [128,512] VectorE relu+bias (fp32 PSUM -> fp16) and one
    [128,512] ScalarE copy (PSUM -> fp16 SBUF) per pair - all
    element-wise work runs on full 128 partitions.
  - Layer-2 + output copy are software-pipelined one pair behind
    layer-1 so the PE queue never stalls on the vector engine.
  - Inputs stream on the sync-engine HWDGE queue, outputs on the
    scalar-engine HWDGE queue (independent FIFOs).
"""

import numpy as np

import concourse.bacc as bacc
import concourse.bass as bass
import concourse.mybir as mybir
import concourse.tile as tile
from concourse.bass_utils import run_bass_kernel_spmd

# ---- problem constants (hardcoded per harness contract) ----
E_TOTAL = 1_000_000
N_CORES = 8
IN_C = 64
IN_E = 32
HID = 64
OUT_C = 64

NHALF = 512                    # edges per 64-col output tile / matmul N
Q_PER_BLK = 8                  # 512-edge tiles per block
P_PER_BLK = Q_PER_BLK // 2     # 4 pairs per block
BLK_EDGES = NHALF * Q_PER_BLK  # 4096
EPC = E_TOTAL // N_CORES       # 125000 edges per core
N_BLK = -(-EPC // BLK_EDGES)   # 31
EPAD = N_BLK * BLK_EDGES       # 126976

ECOLS = BLK_EDGES // 4         # 1024 e-columns per block (32-row bands)
XBASE = ECOLS                  # x-columns start after the e-columns
INCOLS = BLK_EDGES + ECOLS     # 5120

F32 = mybir.dt.float32
F16 = mybir.dt.float16

# test.py hooks
_TRACE = False
LAST_RESULT = None

_PROGRAM_CACHE = {}


def _build_program():
    nc = bacc.Bacc(
        "TRN2",
        target_bir_lowering=False,
        debug=False,
        num_devices=N_CORES,
    )

    xin = nc.declare_dram_parameter(
        "xin", [N_BLK, 128, INCOLS], F16, isOutput=False
    )
    wx = nc.declare_dram_parameter("wx", [128, HID], F16, isOutput=False)
    wes2d = nc.declare_dram_parameter("wes2d", [128, 128], F16, isOutput=False)
    w2d = nc.declare_dram_parameter("w2d", [128, 128], F16, isOutput=False)
    b1r = nc.declare_dram_parameter("b1r", [128, 1], F32, isOutput=False)
    out = nc.declare_dram_parameter(
        "out", [N_BLK, 128, P_PER_BLK * NHALF], F16, isOutput=True
    )

    with tile.TileContext(nc) as tc:
        with (
            tc.tile_pool(name="consts", bufs=1) as cpool,
            tc.tile_pool(name="xi", bufs=4) as xi_pool,
            tc.tile_pool(name="hh", bufs=5) as hh_pool,
            tc.tile_pool(name="ob", bufs=3) as ob_pool,
            tc.tile_pool(name="ph", bufs=4, space="PSUM") as ph_pool,
            tc.tile_pool(name="po", bufs=4, space="PSUM") as po_pool,
        ):
            wx_t = cpool.tile([128, HID], F16)
            nc.sync.dma_start(wx_t[:], wx[:])
            wes2d_t = cpool.tile([128, 128], F16)
            nc.sync.dma_start(wes2d_t[:], wes2d[:])
            w2d_t = cpool.tile([128, 128], F16)
            nc.sync.dma_start(w2d_t[:], w2d[:])
            b1r_t = cpool.tile([128, 1], F32)
            nc.sync.dma_start(b1r_t[:], b1r[:])

            # Warm the PE clock gate (HAM): a dense block of full-array
            # matmuls reliably raises the PE clock 1.2 -> 2.4 GHz ~7us in
            # (the quadrant-tiled real stream alone never triggers the
            # raise, even when gap-free). The raised clock then sticks as
            # long as the real stream avoids >~1.3us PE stalls.
            warm_t = cpool.tile([128, NHALF], F16)
            nc.vector.memset(warm_t[:], 0.0)
            warm_ps = ph_pool.tile([128, NHALF], F32, tag="ph_t", name="warm_ps")
            for _ in range(12):
                nc.tensor.matmul(
                    warm_ps[:, :], warm_t[:, 0:128], warm_t[:, :],
                    start=True, stop=True,
                )

            # software pipeline: layer-2 runs THREE pairs behind layer-1
            # so the PE queue neither waits on the vector engine's
            # relu+sem latency nor on input-DMA jitter.
            # entries: (hh tile, ob tile, pair idx, blk)
            pending = []

            def emit_l2(p):
                hh, ob_t, pr, b = p
                po = po_pool.tile([128, NHALF], F32, tag="po_t", name="po")
                nc.tensor.matmul(
                    po[:, :], w2d_t[:, :], hh[:, :],
                    start=True, stop=True, tile_position=(0, 0),
                )
                nc.scalar.activation(
                    ob_t[:, pr * NHALF : (pr + 1) * NHALF], po[:, :],
                    mybir.ActivationFunctionType.Copy,
                )
                if pr == P_PER_BLK - 1:
                    nc.scalar.dma_start(out[b], ob_t[:])

            for blk in range(N_BLK):
                xi_t = xi_pool.tile([128, INCOLS], F16)
                # chunked input DMA (e-columns first): pair p only waits
                # on its own half-block chunk, so completion semaphores
                # pace ahead of PE consumption. Block 0 is split finest
                # to hand off from warmup with no PE idle gap.
                if blk == 0:
                    for ck in range(5):
                        c0 = ck * ECOLS
                        nc.sync.dma_start(
                            xi_t[:, c0 : c0 + ECOLS],
                            xin[blk, :, c0 : c0 + ECOLS],
                        )
                else:
                    nc.sync.dma_start(xi_t[:, 0:ECOLS], xin[blk, :, 0:ECOLS])
                    half = (INCOLS - ECOLS) // 2
                    nc.sync.dma_start(
                        xi_t[:, ECOLS : ECOLS + half],
                        xin[blk, :, ECOLS : ECOLS + half],
                    )
                    nc.sync.dma_start(
                        xi_t[:, ECOLS + half : INCOLS],
                        xin[blk, :, ECOLS + half : INCOLS],
                    )
                ob_t = ob_pool.tile([128, P_PER_BLK * NHALF], F16)

                for pr in range(P_PER_BLK):
                    # tiles qa = 2*pr, qb = 2*pr+1 -> PSUM rows 0:64 /
                    # 64:128; both e-tiles sit stacked in one 64-row band
                    # (rows 64*(pr%2)..+64, cols 512*(pr//2)), so ONE
                    # K=64 full-width matmul with blockdiag(We, We) adds
                    # both e contributions.
                    qa, qb = 2 * pr, 2 * pr + 1
                    er = 64 * (pr % 2)
                    ec = NHALF * (pr // 2)
                    ph = ph_pool.tile([128, NHALF], F32, tag="ph_t", name="ph")
                    nc.tensor.matmul(
                        ph[0:64, :], wx_t[:, :],
                        xi_t[:, XBASE + qa * NHALF : XBASE + (qa + 1) * NHALF],
                        start=True, stop=False, tile_position=(0, 0),
                    )
                    nc.tensor.matmul(
                        ph[64:128, :], wx_t[:, :],
                        xi_t[:, XBASE + qb * NHALF : XBASE + (qb + 1) * NHALF],
                        start=True, stop=False, tile_position=(0, 64),
                    )
                    nc.tensor.matmul(
                        ph[:, :],
                        wes2d_t[er : er + 64, :],
                        xi_t[er : er + 64, ec : ec + NHALF],
                        start=False, stop=True, tile_position=(er, 0),
                        skip_group_check=True,
                    )
                    # relu(ph + b1) -> fp16, full 128 partitions
                    hh = hh_pool.tile([128, NHALF], F16, tag="hh", name="hh")
                    nc.vector.tensor_scalar(
                        hh[:, :], ph[:, :], b1r_t[:, :], 0.0,
                        mybir.AluOpType.add, mybir.AluOpType.max,
                    )
                    # layer 2 from three pairs ago (software pipelining)
                    if len(pending) == 3:
                        emit_l2(pending.pop(0))
                    pending.append((hh, ob_t, pr, blk))

            for p in pending:
                emit_l2(p)

    nc.compile()
    return nc


def _get_program():
    if "prog" not in _PROGRAM_CACHE:
        _PROGRAM_CACHE["prog"] = _build_program()
    return _PROGRAM_CACHE["prog"]


def _pad_rows(a, n):
    if a.shape[0] == n:
        return a
    pad = np.zeros((n - a.shape[0],) + a.shape[1:], dtype=a.dtype)
    return np.concatenate([a, pad], axis=0)


def _host_pack(v_i, v_j, e_ij, W1, b1, W2, b2):
    """Build per-core input maps in the device layouts."""
    W1 = np.asarray(W1, dtype=np.float32)
    W2 = np.asarray(W2, dtype=np.float32)
    wx_h = W1[:128].astype(np.float16)
    wes_h = W1[128:160].astype(np.float16)
    w2_h = W2.astype(np.float16)

    w2d = np.zeros((128, 128), dtype=np.float16)
    w2d[0:64, 0:64] = w2_h
    w2d[64:128, 64:128] = w2_h

    # blockdiag(We, We) [64, 128], tiled twice down the partitions so the
    # e-matmul's stationary operand sits at the same base partition as its
    # moving band (rows 0:64 or 64:128).
    wes2d_half = np.zeros((64, 128), dtype=np.float16)
    wes2d_half[0:32, 0:64] = wes_h
    wes2d_half[32:64, 64:128] = wes_h
    wes2d = np.tile(wes2d_half, (2, 1))

    weights = {
        "wx": np.ascontiguousarray(wx_h),
        "wes2d": np.ascontiguousarray(wes2d),
        "w2d": w2d,
        "b1r": np.ascontiguousarray(np.tile(b1, 2)[:, None], dtype=np.float32),
    }

    in_maps = []
    for c in range(N_CORES):
        sl = slice(c * EPC, (c + 1) * EPC)
        vi = _pad_rows(np.asarray(v_i[sl], dtype=np.float16), EPAD)
        vj = _pad_rows(np.asarray(v_j[sl], dtype=np.float16), EPAD)
        ec = _pad_rows(np.asarray(e_ij[sl], dtype=np.float16), EPAD)

        # x-part: [vi^T; vj^T] -> [N_BLK, 128, 4096]
        X = np.concatenate([vi.T, vj.T], axis=0)          # [128, EPAD] f16
        xa = X.reshape(128, N_BLK, BLK_EDGES).transpose(1, 0, 2)

        # e-part: tile q = 4h + i -> rows 32i:32i+32, cols 512h:512h+512
        ET = ec.T                                          # [32, EPAD] f16
        ebd = ET.reshape(32, N_BLK, 2, 4, NHALF).transpose(1, 3, 0, 2, 4)
        ebd = ebd.reshape(N_BLK, 128, ECOLS)               # [blk, 32i+r, 512h+n]

        xi_full = np.concatenate([ebd, xa], axis=2)        # [N_BLK, 128, 5120]
        in_maps.append({"xin": np.ascontiguousarray(xi_full), **weights})
    return in_maps


def _host_unpack(results, b2):
    """results: per-core dicts with 'out' [N_BLK, 128, 2048] f16."""
    b2 = np.asarray(b2, dtype=np.float32)
    outs = []
    for c in range(N_CORES):
        o = np.asarray(results[c]["out"])
        # o[blk, 64r + j, 512p + n] = OUT[blk*4096 + (2p + r)*512 + n, j]
        r = o.reshape(N_BLK, 2, 64, P_PER_BLK, NHALF)  # [blk, r, j, p, n]
        r = r.transpose(0, 3, 1, 4, 2)                  # [blk, p, r, n, j]
        r = np.ascontiguousarray(r).reshape(EPAD, OUT_C)[:EPC]
        outs.append(r.astype(np.float32) + b2)
    return np.concatenate(outs, axis=0)


def kernel(v_i, v_j, e_ij, W1, b1, W2, b2):
    global LAST_RESULT
    nc = _get_program()
    in_maps = _host_pack(v_i, v_j, e_ij, W1, b1, W2, b2)
    res = run_bass_kernel_spmd(
        nc, in_maps, core_ids=list(range(N_CORES)), trace=_TRACE
    )
    LAST_RESULT = res
    return _host_unpack(res.results, b2)


# revision 24
# speedup vs baseline: 2.9338x; 1.0724x over previous
"""Trainium2 Bass kernel for the GNN message-update MLP:

    out = relu(concat([v_i, v_j, e_ij], -1) @ W1 + b1) @ W2 + b2

Strategy (memory-bound, E = 1M edges, data-parallel across 8 cores):
  - Shard edges across the 8 NeuronCores (125000 each, padded to 126976).
  - Pure fp16 I/O: activations ship as fp16 (half the HBM bytes of fp32)
    and the output is written back as fp16, converted to fp32 on host.
    PSUM accumulation stays fp32; end-to-end error ~5e-4 of scale.
  - Per 1024-edge pair (two 512-edge tiles on PSUM row halves via column
    tile_position): 2x K=128 x-matmuls + 2x K=32 e-matmuls (e rows are
    partition-stacked so each e-matmul streams from its own 32-row band)
    + ONE full-width layer-2 matmul with a block-diagonal [W2 0; 0 W2]
    stationary operand. 5 matmuls / 1024 edges, all N=512.
  - One [128,512] VectorE relu+bias (fp32 PSUM -> fp16) and one
    [128,512] ScalarE copy (PSUM -> fp16 SBUF) per pair - all
    element-wise work runs on full 128 partitions.
  - Layer-2 + output copy are software-pipelined one pair behind
    layer-1 so the PE queue never stalls on the vector engine.
  - Inputs stream on the sync-engine HWDGE queue, outputs on the
    scalar-engine HWDGE queue (independent FIFOs).
"""

import numpy as np

import concourse.bacc as bacc
import concourse.bass as bass
import concourse.mybir as mybir
import concourse.tile as tile
from concourse.bass_utils import run_bass_kernel_spmd

# ---- problem constants (hardcoded per harness contract) ----
E_TOTAL = 1_000_000
N_CORES = 8
IN_C = 64
IN_E = 32
HID = 64
OUT_C = 64

NHALF = 512                    # edges per 64-col output tile / matmul N
Q_PER_BLK = 8                  # 512-edge tiles per block
P_PER_BLK = Q_PER_BLK // 2     # 4 pairs per block
BLK_EDGES = NHALF * Q_PER_BLK  # 4096
EPC = E_TOTAL // N_CORES       # 125000 edges per core
N_BLK = -(-EPC // BLK_EDGES)   # 31
EPAD = N_BLK * BLK_EDGES       # 126976
# pairs per block: full blocks have 4; the tail block only covers the
# 2120 leftover edges -> 3 pairs (3072 edges), trimming pad DMA+compute
P_LAST = -(-(EPC - (N_BLK - 1) * BLK_EDGES) // (2 * NHALF))  # 3

ECOLS = BLK_EDGES // 4         # 1024 e-columns per block (32-row bands)
XBASE = ECOLS                  # x-columns start after the e-columns
INCOLS = BLK_EDGES + ECOLS     # 5120

F32 = mybir.dt.float32
F16 = mybir.dt.float16

# test.py hooks
_TRACE = False
LAST_RESULT = None

_PROGRAM_CACHE = {}


def _build_program():
    nc = bacc.Bacc(
        "TRN2",
        target_bir_lowering=False,
        debug=False,
        num_devices=N_CORES,
    )

    xin = nc.declare_dram_parameter(
        "xin", [N_BLK, 128, INCOLS], F16, isOutput=False
    )
    wx = nc.declare_dram_parameter("wx", [128, HID], F16, isOutput=False)
    wes2d = nc.declare_dram_parameter("wes2d", [128, 128], F16, isOutput=False)
    w2d = nc.declare_dram_parameter("w2d", [128, 128], F16, isOutput=False)
    b1r = nc.declare_dram_parameter("b1r", [128, 1], F32, isOutput=False)
    out = nc.declare_dram_parameter(
        "out", [N_BLK, 128, P_PER_BLK * NHALF], F16, isOutput=True
    )

    with tile.TileContext(nc) as tc:
        with (
            tc.tile_pool(name="consts", bufs=1) as cpool,
            tc.tile_pool(name="xi", bufs=4) as xi_pool,
            tc.tile_pool(name="hh", bufs=5) as hh_pool,
            tc.tile_pool(name="ob", bufs=3) as ob_pool,
            tc.tile_pool(name="ph", bufs=4, space="PSUM") as ph_pool,
            tc.tile_pool(name="po", bufs=4, space="PSUM") as po_pool,
        ):
            wx_t = cpool.tile([128, HID], F16)
            wes2d_t = cpool.tile([128, 128], F16)
            w2d_t = cpool.tile([128, 128], F16)
            b1r_t = cpool.tile([128, 1], F32)

            # Warm the PE clock gate (HAM): a dense block of full-array
            # matmuls reliably raises the PE clock 1.2 -> 2.4 GHz ~7us in
            # (the quadrant-tiled real stream alone never triggers the
            # raise, even when gap-free). The raised clock then sticks as
            # long as the real stream avoids >~1.3us PE stalls.
            warm_t = cpool.tile([128, NHALF], F16)
            nc.vector.memset(warm_t[:], 0.0)
            warm_ps = ph_pool.tile([128, NHALF], F32, tag="ph_t", name="warm_ps")
            for _ in range(12):
                nc.tensor.matmul(
                    warm_ps[:, :], warm_t[:, 0:128], warm_t[:, :],
                    start=True, stop=True,
                )

            # software pipeline: layer-2 runs THREE pairs behind layer-1
            # so the PE queue neither waits on the vector engine's
            # relu+sem latency nor on input-DMA jitter.
            # entries: (hh tile, ob tile, pair idx, blk)
            pending = []

            def emit_l2(p):
                hh, ob_t, pr, b, npr = p
                po = po_pool.tile([128, NHALF], F32, tag="po_t", name="po")
                nc.tensor.matmul(
                    po[:, :], w2d_t[:, :], hh[:, :],
                    start=True, stop=True, tile_position=(0, 0),
                )
                nc.scalar.activation(
                    ob_t[:, pr * NHALF : (pr + 1) * NHALF], po[:, :],
                    mybir.ActivationFunctionType.Copy,
                )
                if pr == npr - 1:
                    nc.scalar.dma_start(
                        out[b, :, 0 : npr * NHALF], ob_t[:, 0 : npr * NHALF]
                    )

            for blk in range(N_BLK):
                xi_t = xi_pool.tile([128, INCOLS], F16)
                # Early blocks: chunked input DMA (e-columns first) so
                # completion semaphores pace ahead of PE consumption and
                # the warmup->real handoff has no PE idle gap. Steady
                # state: one big DMA per block (fewer packets -> better
                # HBM efficiency); the 4-5 block lookahead hides the
                # completion latency.
                if blk == 0:
                    for ck in range(2):
                        c0 = ck * ECOLS
                        nc.sync.dma_start(
                            xi_t[:, c0 : c0 + ECOLS],
                            xin[blk, :, c0 : c0 + ECOLS],
                        )
                    # weights ride after the first two chunks: needed
                    # just before the first real matmul
                    nc.sync.dma_start(wx_t[:], wx[:])
                    nc.sync.dma_start(wes2d_t[:], wes2d[:])
                    nc.sync.dma_start(w2d_t[:], w2d[:])
                    nc.sync.dma_start(b1r_t[:], b1r[:])
                    for ck in range(2, 5):
                        c0 = ck * ECOLS
                        nc.sync.dma_start(
                            xi_t[:, c0 : c0 + ECOLS],
                            xin[blk, :, c0 : c0 + ECOLS],
                        )
                elif blk <= 2:
                    nc.sync.dma_start(xi_t[:, 0:ECOLS], xin[blk, :, 0:ECOLS])
                    half = (INCOLS - ECOLS) // 2
                    nc.sync.dma_start(
                        xi_t[:, ECOLS : ECOLS + half],
                        xin[blk, :, ECOLS : ECOLS + half],
                    )
                    nc.sync.dma_start(
                        xi_t[:, ECOLS + half : INCOLS],
                        xin[blk, :, ECOLS + half : INCOLS],
                    )
                else:
                    npr = P_LAST if blk == N_BLK - 1 else P_PER_BLK
                    ncols = ECOLS + npr * 2 * NHALF
                    nc.sync.dma_start(
                        xi_t[:, 0:ncols], xin[blk, :, 0:ncols]
                    )
                ob_t = ob_pool.tile([128, P_PER_BLK * NHALF], F16)

                n_pairs = P_LAST if blk == N_BLK - 1 else P_PER_BLK
                for pr in range(n_pairs):
                    # tiles qa = 2*pr, qb = 2*pr+1 -> PSUM rows 0:64 /
                    # 64:128; both e-tiles sit stacked in one 64-row band
                    # (rows 64*(pr%2)..+64, cols 512*(pr//2)), so ONE
                    # K=64 full-width matmul with blockdiag(We, We) adds
                    # both e contributions.
                    qa, qb = 2 * pr, 2 * pr + 1
                    er = 64 * (pr % 2)
                    ec = NHALF * (pr // 2)
                    ph = ph_pool.tile([128, NHALF], F32, tag="ph_t", name="ph")
                    nc.tensor.matmul(
                        ph[0:64, :], wx_t[:, :],
                        xi_t[:, XBASE + qa * NHALF : XBASE + (qa + 1) * NHALF],
                        start=True, stop=False, tile_position=(0, 0),
                    )
                    nc.tensor.matmul(
                        ph[64:128, :], wx_t[:, :],
                        xi_t[:, XBASE + qb * NHALF : XBASE + (qb + 1) * NHALF],
                        start=True, stop=False, tile_position=(0, 64),
                    )
                    nc.tensor.matmul(
                        ph[:, :],
                        wes2d_t[er : er + 64, :],
                        xi_t[er : er + 64, ec : ec + NHALF],
                        start=False, stop=True, tile_position=(er, 0),
                        skip_group_check=True,
                    )
                    # relu(ph + b1) -> fp16, full 128 partitions
                    hh = hh_pool.tile([128, NHALF], F16, tag="hh", name="hh")
                    nc.vector.tensor_scalar(
                        hh[:, :], ph[:, :], b1r_t[:, :], 0.0,
                        mybir.AluOpType.add, mybir.AluOpType.max,
                    )
                    # layer 2 from three pairs ago (software pipelining)
                    if len(pending) == 3:
                        emit_l2(pending.pop(0))
                    pending.append((hh, ob_t, pr, blk, n_pairs))

            for p in pending:
                emit_l2(p)

    nc.compile()
    return nc


def _get_program():
    if "prog" not in _PROGRAM_CACHE:
        _PROGRAM_CACHE["prog"] = _build_program()
    return _PROGRAM_CACHE["prog"]


def _pad_rows(a, n):
    if a.shape[0] == n:
        return a
    pad = np.zeros((n - a.shape[0],) + a.shape[1:], dtype=a.dtype)
    return np.concatenate([a, pad], axis=0)


def _host_pack(v_i, v_j, e_ij, W1, b1, W2, b2):
    """Build per-core input maps in the device layouts."""
    W1 = np.asarray(W1, dtype=np.float32)
    W2 = np.asarray(W2, dtype=np.float32)
    wx_h = W1[:128].astype(np.float16)
    wes_h = W1[128:160].astype(np.float16)
    w2_h = W2.astype(np.float16)

    w2d = np.zeros((128, 128), dtype=np.float16)
    w2d[0:64, 0:64] = w2_h
    w2d[64:128, 64:128] = w2_h

    # blockdiag(We, We) [64, 128], tiled twice down the partitions so the
    # e-matmul's stationary operand sits at the same base partition as its
    # moving band (rows 0:64 or 64:128).
    wes2d_half = np.zeros((64, 128), dtype=np.float16)
    wes2d_half[0:32, 0:64] = wes_h
    wes2d_half[32:64, 64:128] = wes_h
    wes2d = np.tile(wes2d_half, (2, 1))

    weights = {
        "wx": np.ascontiguousarray(wx_h),
        "wes2d": np.ascontiguousarray(wes2d),
        "w2d": w2d,
        "b1r": np.ascontiguousarray(np.tile(b1, 2)[:, None], dtype=np.float32),
    }

    in_maps = []
    for c in range(N_CORES):
        sl = slice(c * EPC, (c + 1) * EPC)
        vi = _pad_rows(np.asarray(v_i[sl], dtype=np.float16), EPAD)
        vj = _pad_rows(np.asarray(v_j[sl], dtype=np.float16), EPAD)
        ec = _pad_rows(np.asarray(e_ij[sl], dtype=np.float16), EPAD)

        # x-part: [vi^T; vj^T] -> [N_BLK, 128, 4096]
        X = np.concatenate([vi.T, vj.T], axis=0)          # [128, EPAD] f16
        xa = X.reshape(128, N_BLK, BLK_EDGES).transpose(1, 0, 2)

        # e-part: tile q = 4h + i -> rows 32i:32i+32, cols 512h:512h+512
        ET = ec.T                                          # [32, EPAD] f16
        ebd = ET.reshape(32, N_BLK, 2, 4, NHALF).transpose(1, 3, 0, 2, 4)
        ebd = ebd.reshape(N_BLK, 128, ECOLS)               # [blk, 32i+r, 512h+n]

        xi_full = np.concatenate([ebd, xa], axis=2)        # [N_BLK, 128, 5120]
        in_maps.append({"xin": np.ascontiguousarray(xi_full), **weights})
    return in_maps


def _host_unpack(results, b2):
    """results: per-core dicts with 'out' [N_BLK, 128, 2048] f16."""
    b2 = np.asarray(b2, dtype=np.float32)
    outs = []
    for c in range(N_CORES):
        o = np.asarray(results[c]["out"])
        # o[blk, 64r + j, 512p + n] = OUT[blk*4096 + (2p + r)*512 + n, j]
        r = o.reshape(N_BLK, 2, 64, P_PER_BLK, NHALF)  # [blk, r, j, p, n]
        r = r.transpose(0, 3, 1, 4, 2)                  # [blk, p, r, n, j]
        r = np.ascontiguousarray(r).reshape(EPAD, OUT_C)[:EPC]
        outs.append(r.astype(np.float32) + b2)
    return np.concatenate(outs, axis=0)


def kernel(v_i, v_j, e_ij, W1, b1, W2, b2):
    global LAST_RESULT
    nc = _get_program()
    in_maps = _host_pack(v_i, v_j, e_ij, W1, b1, W2, b2)
    res = run_bass_kernel_spmd(
        nc, in_maps, core_ids=list(range(N_CORES)), trace=_TRACE
    )
    LAST_RESULT = res
    return _host_unpack(res.results, b2)
